# revision 2
# baseline (speedup 1.0000x reference)
"""Trainium2 Bass kernel for nn_DGNN (TGN-style dual-memory GNN message passing).

Strategy (8 NeuronCores, SPMD):
  - Edges sharded across cores (1024/core). Per-core "slots" = [src; dst; neg]
    (3072 rows) are the only memory rows whose updated values the output needs.
  - Each core: indirect-gathers memory/edge-feature rows, computes merges,
    messages, GRU deltas and neighbor-prop vectors (the "T table") for its own
    edges; T rows needed by other cores are exchanged with one AllToAll per
    side (local rows bypass the collective entirely).
  - Scatter-add is a budgeted segment-sum: gather contribution rows from the
    inbox in slot order, multiply by host-built Sel blocks on the TensorEngine
    (PSUM accumulation), add the gathered base memory rows.
  - Scores: merge + bilinear projections + row-dot + sigmoid, all on-device.

Host side does index routing only (is_last flags, decay weights folded into
Sel, A2A packing plans); all tensor math and memory-row traffic is on-device.
"""
import sys

sys.path.insert(0, "/opt/trn_rl_repo")

import numpy as np

import concourse.bass as bass
import concourse.tile as tile
from concourse import bacc, mybir
from concourse.bass_utils import run_bass_kernel_spmd
from concourse.masks import make_identity

F32 = mybir.dt.float32
BF16 = mybir.dt.bfloat16
I32 = mybir.dt.int32
AF = mybir.ActivationFunctionType
ALU = mybir.AluOpType

N, M, DM, DE, E, K, NE = 200000, 200, 100, 172, 8192, 10, 400000
TAU = 2.0
C = 8
EC = E // C          # 1024 edges per core
S = 3 * EC           # 3072 slots per core
OT = S // 128        # 24 output slot tiles
ECH = EC // 128      # 8 edge chunks
P = 128

TRACE = False
LAST_RESULT = None

# fixed (input-independent) exchange/budget sizes; host prep asserts they fit
NSP = 256            # A2A rows per (owner, receiver) pair, off-diagonal
Bx = 2               # X tiles (128 contributions) per output slot tile
NP = (C * NSP) // P  # pack gather column count (16)
IB = 2 * EC + C * NSP  # inbox rows: local T table + A2A output


# --------------------------------------------------------------------------
# host-side preparation (index routing)
# --------------------------------------------------------------------------

def _is_last(nodes):
    order = np.arange(E, dtype=np.int64)
    last_occ = np.full(N, -1, dtype=np.int64)
    np.maximum.at(last_occ, nodes, order)
    return last_occ[nodes] == order


def _prep(inputs):
    src = np.asarray(inputs["source_nodes"]).astype(np.int64)
    dst = np.asarray(inputs["destination_nodes"]).astype(np.int64)
    neg = np.asarray(inputs["negative_nodes"]).astype(np.int64)
    eidx = np.asarray(inputs["edge_idxs"]).astype(np.int64)
    times = np.asarray(inputs["edge_times"]).astype(np.float64)
    nbrs = np.asarray(inputs["neighbors"]).astype(np.int64)
    lu = {
        "s": np.asarray(inputs["last_update_s"]).astype(np.float64),
        "g": np.asarray(inputs["last_update_g"]).astype(np.float64),
    }
    endp = {"s": src, "g": dst}
    out = {}

    slots = np.zeros((C, S), dtype=np.int64)
    for c in range(C):
        slots[c] = np.concatenate(
            [src[c * EC:(c + 1) * EC], dst[c * EC:(c + 1) * EC],
             neg[c * EC:(c + 1) * EC]])
    out["slots"] = slots
    out["slots_idx"] = np.transpose(
        slots.reshape(C, OT, P), (0, 2, 1)).astype(np.int32).copy()
    out["ef_idx"] = np.transpose(
        eidx.reshape(C, ECH, P), (0, 2, 1)).astype(np.int32).copy()

    flat_nodes = slots.reshape(-1)
    sort_perm = np.argsort(flat_nodes, kind="stable")
    sorted_nodes = flat_nodes[sort_perm]

    is_last = {"s": _is_last(src), "g": _is_last(dst)}

    sides = {}
    for side in ("s", "g"):
        nodes = endp[side]
        le = np.where(is_last[side])[0]
        nb = nbrs[nodes[le]]
        dt = times[le, None] - lu[side][nb]
        decay = np.exp(-np.clip(dt, 0.0, 50.0) / TAU)
        edge = np.concatenate([le, np.repeat(le, K)])
        kind = np.concatenate([np.zeros(len(le), np.int64),
                               np.ones(len(le) * K, np.int64)])
        tgt = np.concatenate([nodes[le], nb.reshape(-1)])
        w = np.concatenate([np.ones(len(le)), decay.reshape(-1)])

        lo = np.searchsorted(sorted_nodes, tgt, side="left")
        hi = np.searchsorted(sorted_nodes, tgt, side="right")
        cnt = hi - lo
        keep = cnt > 0
        lo, cnt = lo[keep], cnt[keep]
        edge, kind, w = edge[keep], kind[keep], w[keep]
        reps = np.repeat(np.arange(len(lo)), cnt)
        pos = (np.arange(len(reps)) -
               np.repeat(np.concatenate([[0], np.cumsum(cnt)[:-1]]), cnt))
        gslot = sort_perm[lo[reps] + pos]
        sides[side] = dict(edge=edge[reps], kind=kind[reps], w=w[reps],
                           core=gslot // S, lslot=gslot % S)

    # A2A pack lists (off-diagonal only)
    pair_rows = {}
    nsp_need = 0
    for side in ("s", "g"):
        sd = sides[side]
        owner = sd["edge"] // EC
        trow = sd["kind"] * EC + (sd["edge"] % EC)
        for o in range(C):
            for r in range(C):
                if o == r:
                    pair_rows[(side, o, r)] = np.zeros(0, dtype=np.int64)
                    continue
                m = (owner == o) & (sd["core"] == r)
                rows = np.unique(trow[m])
                pair_rows[(side, o, r)] = rows
                nsp_need = max(nsp_need, len(rows))
    assert nsp_need <= NSP, f"NSP {NSP} too small, need {nsp_need}"

    pack_idx = {}
    for side in ("s", "g"):
        pk = np.zeros((C, C * NSP), dtype=np.int32)
        for o in range(C):
            for r in range(C):
                rows = pair_rows[(side, o, r)]
                pk[o, r * NSP:r * NSP + len(rows)] = rows
        pack_idx[side] = np.transpose(
            pk.reshape(C, NP, P), (0, 2, 1)).copy()
    out["pack_idx"] = pack_idx

    xg_idx = {k: np.zeros((C, P, OT * Bx), dtype=np.int32) for k in "sg"}
    sel = {k: np.zeros((C, P, OT * Bx * P), dtype=np.float32) for k in "sg"}
    for side in ("s", "g"):
        sd = sides[side]
        owner = sd["edge"] // EC
        trow = sd["kind"] * EC + (sd["edge"] % EC)
        for r in range(C):
            m = sd["core"] == r
            ow, tr = owner[m], trow[m]
            ls, ww = sd["lslot"][m], sd["w"][m]
            ib = np.zeros(len(ow), dtype=np.int64)
            for o in range(C):
                mo = ow == o
                if o == r:
                    ib[mo] = tr[mo]
                else:
                    rows = pair_rows[(side, o, r)]
                    ib[mo] = 2 * EC + o * NSP + np.searchsorted(rows, tr[mo])
            otile = ls // P
            order = np.argsort(otile, kind="stable")
            ib, ot_, tt, ww = ib[order], otile[order], (ls % P)[order], ww[order]
            for o in range(OT):
                mm = ot_ == o
                ibo, tto, wwo = ib[mm], tt[mm], ww[mm]
                assert len(ibo) <= Bx * P, f"Bx {Bx} too small: {len(ibo)}"
                j = np.arange(len(ibo))
                b, i = j // P, j % P
                xg_idx[side][r, i, o * Bx + b] = ibo
                np.add.at(sel[side][r], (i, (o * Bx + b) * P + tto), wwo)
    out["xg_idx"] = xg_idx
    out["sel"] = sel

    W_merge = np.asarray(inputs["W_merge"], dtype=np.float32)
    b_merge = np.asarray(inputs["b_merge"], dtype=np.float32)
    W_msg = np.asarray(inputs["W_msg"], dtype=np.float32)
    b_msg = np.asarray(inputs["b_msg"], dtype=np.float32)
    out["Wmerge_d"] = np.concatenate([W_merge, b_merge[None, :]], axis=0)
    out["Wmsg_d"] = np.concatenate([W_msg, b_msg[None, :]], axis=0)
    for side in ("s", "g"):
        Wx = np.asarray(inputs[f"Wx_{side}"], dtype=np.float32)
        bg = np.asarray(inputs[f"bg_{side}"], dtype=np.float32)
        out[f"Wx_{side}_d"] = np.concatenate([Wx, bg[None, :]], axis=0)
        out[f"Wh_{side}_d"] = np.asarray(inputs[f"Wh_{side}"], dtype=np.float32)
        out[f"Wprop_{side}_d"] = np.asarray(inputs[f"W_prop_{side}"],
                                            dtype=np.float32)
    out["Ws_d"] = np.asarray(inputs["W_s"], dtype=np.float32)
    out["Wg_d"] = np.asarray(inputs["W_g"], dtype=np.float32)
    return out


# --------------------------------------------------------------------------
# device kernel
# --------------------------------------------------------------------------

def _build():
    nc = bacc.Bacc("TRN2", target_bir_lowering=False, debug=False,
                   num_devices=C)

    mem_s_d = nc.dram_tensor("mem_s", [N, M], F32, kind="ExternalInput")
    mem_g_d = nc.dram_tensor("mem_g", [N, M], F32, kind="ExternalInput")
    ef_d = nc.dram_tensor("ef_tab", [NE, DE], F32, kind="ExternalInput")
    slots_idx_d = nc.dram_tensor("slots_idx", [P, OT], I32, kind="ExternalInput")
    ef_idx_d = nc.dram_tensor("ef_idx", [P, ECH], I32, kind="ExternalInput")
    pack_idx_d = {s: nc.dram_tensor(f"pack_idx_{s}", [P, NP], I32,
                                    kind="ExternalInput") for s in "sg"}
    xg_idx_d = {s: nc.dram_tensor(f"xg_idx_{s}", [P, OT * Bx], I32,
                                  kind="ExternalInput") for s in "sg"}
    sel_d = {s: nc.dram_tensor(f"sel_{s}", [P, OT * Bx * P], BF16,
                               kind="ExternalInput") for s in "sg"}
    wm_d = nc.dram_tensor("Wmerge", [401, M], F32, kind="ExternalInput")
    wmsg_d = nc.dram_tensor("Wmsg", [573, DM], F32, kind="ExternalInput")
    wx_d = {s: nc.dram_tensor(f"Wx_{s}", [101, 600], F32, kind="ExternalInput")
            for s in "sg"}
    wh_d = {s: nc.dram_tensor(f"Wh_{s}", [200, 600], F32, kind="ExternalInput")
            for s in "sg"}
    wprop_d = {s: nc.dram_tensor(f"Wprop_{s}", [100, 200], F32,
                                 kind="ExternalInput") for s in "sg"}
    ws_d = nc.dram_tensor("Ws", [200, 100], F32, kind="ExternalInput")
    wg_d = nc.dram_tensor("Wg", [200, 100], F32, kind="ExternalInput")
    ones_d = nc.dram_tensor("ones", [1, 4096], F32, kind="ExternalInput")
    out_d = nc.dram_tensor("out", [2, EC], F32, kind="ExternalOutput")

    with tile.TileContext(nc) as tc:
        evac_ctr = [0]

        def evac(dst_ap, src_ap):
            # alternate PSUM->SBUF evacuation between DVE and ACT
            if evac_ctr[0] % 2 == 0:
                nc.vector.tensor_copy(dst_ap, src_ap)
            else:
                nc.scalar.copy(dst_ap, src_ap)
            evac_ctr[0] += 1

        with (
            tc.tile_pool(name="const", bufs=1) as cp,
            tc.tile_pool(name="psum", bufs=1, space="PSUM") as pp,
            tc.tile_pool(name="dram", bufs=1, space="DRAM") as dp,
            tc.tile_pool(name="glob", bufs=1) as gp,
        ):
            # ---------------- constants / weights ----------------
            ident = cp.tile([P, P], F32)
            make_identity(nc, ident[:])
            ones = cp.tile([1, 4096], F32)
            nc.sync.dma_start(ones[:1, :], ones_d[:, :])

            def load_rows(dram, r0, r1, cols, name):
                t = cp.tile([r1 - r0, cols], F32, name=name)
                nc.sync.dma_start(t[:, :], dram[r0:r1, :])
                return t

            wm = [load_rows(wm_d, kc * 100, kc * 100 + 100 + (kc == 3), M,
                            f"wm{kc}") for kc in range(4)]
            wmsg = [load_rows(wmsg_d, kc * 100,
                              min(kc * 100 + 100, 572) + (kc == 5), DM,
                              f"wmsg{kc}") for kc in range(6)]
            wx = {s: load_rows(wx_d[s], 0, 101, 600, f"wx{s}") for s in "sg"}
            wh = {s: [load_rows(wh_d[s], kc * 100, kc * 100 + 100, 600,
                                f"wh{s}{kc}") for kc in range(2)] for s in "sg"}
            wprop = {s: load_rows(wprop_d[s], 0, 100, 200, f"wprop{s}")
                     for s in "sg"}
            ws = [load_rows(ws_d, kc * 100, kc * 100 + 100, 100, f"ws{kc}")
                  for kc in range(2)]
            wg = [load_rows(wg_d, kc * 100, kc * 100 + 100, 100, f"wg{kc}")
                  for kc in range(2)]

            slots_idx = cp.tile([P, OT], I32)
            nc.sync.dma_start(slots_idx[:], slots_idx_d[:, :])
            ef_idx = cp.tile([P, ECH], I32)
            nc.sync.dma_start(ef_idx[:], ef_idx_d[:, :])

            # ---------------- slot-row gathers ----------------
            g_sb = {}
            for s, tab in (("s", mem_s_d), ("g", mem_g_d)):
                g = gp.tile([P, OT * M], F32, name=f"g_{s}")
                for j in range(OT):
                    nc.gpsimd.indirect_dma_start(
                        out=g[:, j * M:(j + 1) * M], out_offset=None,
                        in_=tab[:, :],
                        in_offset=bass.IndirectOffsetOnAxis(
                            ap=slots_idx[:, j:j + 1], axis=0))
                g_sb[s] = g

            # inbox DRAM (local T rows + A2A output), per side
            inbox = {s: dp.tile([IB, M], BF16, name=f"inbox_{s}") for s in "sg"}
            a2a_in = {s: dp.tile([C * NSP, M], BF16, name=f"a2a_in_{s}")
                      for s in "sg"}

            # mem2 (base + scatter delta), filled in phase 2
            mem2 = {s: gp.tile([P, OT * M], F32, name=f"mem2_{s}") for s in "sg"}

            # ================= phase 1: per-edge compute =================
            with tc.tile_pool(name="ph1", bufs=1) as p1:
                ef_t = p1.tile([P, ECH * M], F32)
                nc.gpsimd.memset(ef_t[:], 0.0)
                for j in range(ECH):
                    nc.gpsimd.indirect_dma_start(
                        out=ef_t[:, j * M:j * M + DE], out_offset=None,
                        in_=ef_d[:, :],
                        in_offset=bass.IndirectOffsetOnAxis(
                            ap=ef_idx[:, j:j + 1], axis=0))

                # FM transposes of gathered rows (src+dst cols only)
                g_fm = {}
                for s in "sg":
                    fa = p1.tile([100, 2 * EC], F32, name=f"gfm_{s}a")
                    nb_ = 101 if s == "g" else 100
                    fb = p1.tile([nb_, 2 * EC], F32, name=f"gfm_{s}b")
                    if s == "g":
                        nc.sync.dma_start(fb[100:101, :], ones_d[:, 0:2 * EC])
                    for j in range(2 * ECH):
                        for half, dstt in ((0, fa), (1, fb)):
                            pt = pp.tile([100, P], F32, tag="tr", bufs=2,
                                         space="PSUM")
                            nc.tensor.transpose(
                                out=pt[:],
                                in_=g_sb[s][:, j * M + half * 100:
                                            j * M + half * 100 + 100],
                                identity=ident[:])
                            evac(dstt[0:100, j * P:(j + 1) * P], pt[:])
                    g_fm[s] = (fa, fb)

                ef_fm_a = p1.tile([100, EC], F32)
                ef_fm_b = p1.tile([73, EC], F32)
                nc.sync.dma_start(ef_fm_b[72:73, :], ones_d[:, 0:EC])
                for j in range(ECH):
                    pt = pp.tile([100, P], F32, tag="tr", bufs=2, space="PSUM")
                    nc.tensor.transpose(out=pt[:],
                                        in_=ef_t[:, j * M:j * M + 100],
                                        identity=ident[:])
                    evac(ef_fm_a[:, j * P:(j + 1) * P], pt[:])
                    pt2 = pp.tile([72, P], F32, tag="tr", bufs=2, space="PSUM")
                    nc.tensor.transpose(out=pt2[:],
                                        in_=ef_t[:, j * M + 100:j * M + DE],
                                        identity=ident[:])
                    evac(ef_fm_b[0:72, j * P:(j + 1) * P], pt2[:])

                # ---- merges (form A): mm_fm = tanh(Wm.T @ [g_s; g_g]) ----
                mm_fm = (p1.tile([100, 2 * EC], F32, name="mmfa"),
                         p1.tile([100, 2 * EC], F32, name="mmfb"))
                rhs_merge = [g_fm["s"][0], g_fm["s"][1], g_fm["g"][0],
                             g_fm["g"][1]]
                for mo in range(2):
                    for bch in range(2 * EC // 512):
                        bsl = slice(bch * 512, (bch + 1) * 512)
                        pm = pp.tile([100, 512], F32, tag="mm", bufs=2,
                                     space="PSUM")
                        for kc in range(4):
                            rt = rhs_merge[kc]
                            nr = rt.shape[0]
                            nc.tensor.matmul(
                                pm[:], lhsT=wm[kc][0:nr, mo * 100:mo * 100 + 100],
                                rhs=rt[0:nr, bsl],
                                start=(kc == 0), stop=(kc == 3))
                        nc.scalar.activation(mm_fm[mo][:, bsl], pm[:], AF.Tanh)

                # ---- messages (form A): relu(Wmsg.T @ [x_m; y_m; ef]) ----
                msg_fm = {}
                for s in "sg":
                    mf = p1.tile([101, EC], F32, name=f"msg_{s}")
                    nc.sync.dma_start(mf[100:101, :], ones_d[:, 0:EC])
                    first, second = (0, EC) if s == "s" else (EC, 0)
                    for bch in range(EC // 512):
                        b0 = bch * 512
                        bsl = slice(b0, b0 + 512)
                        pm = pp.tile([100, 512], F32, tag="mm", bufs=2,
                                     space="PSUM")
                        rhss = [
                            mm_fm[0][:, first + b0:first + b0 + 512],
                            mm_fm[1][:, first + b0:first + b0 + 512],
                            mm_fm[0][:, second + b0:second + b0 + 512],
                            mm_fm[1][:, second + b0:second + b0 + 512],
                            ef_fm_a[:, bsl], ef_fm_b[:, bsl],
                        ]
                        for kc in range(6):
                            nr = rhss[kc].shape[0]
                            nc.tensor.matmul(pm[:], lhsT=wmsg[kc][0:nr, :],
                                             rhs=rhss[kc],
                                             start=(kc == 0), stop=(kc == 5))
                        nc.scalar.activation(mf[0:100, bsl], pm[:], AF.Relu)
                    msg_fm[s] = mf

                # ---- GRU + prop -> T table (bf16), DMA to inbox ----
                for s in "sg":
                    t_sb = p1.tile([P, 2 * ECH * M], BF16, name=f"t_{s}")
                    hcol = 0 if s == "s" else EC        # h = own endpoint cols
                    htile0 = 0 if s == "s" else ECH     # h BM tile offset in g
                    fa, fb = g_fm[s]
                    for ch in range(ECH):
                        mslc = slice(ch * P, (ch + 1) * P)
                        hslc = slice(hcol + ch * P, hcol + (ch + 1) * P)
                        msg_l = msg_fm[s][0:101, mslc]
                        msg_l100 = msg_fm[s][0:100, mslc]
                        ha = fa[0:100, hslc]
                        hb = fb[0:100, hslc]
                        pr = pp.tile([P, 200], F32, tag="gru", bufs=4,
                                     space="PSUM")
                        pz = pp.tile([P, 200], F32, tag="gru", bufs=4,
                                     space="PSUM")
                        pnx = pp.tile([P, 200], F32, tag="gru", bufs=4,
                                      space="PSUM")
                        pnh = pp.tile([P, 200], F32, tag="gru", bufs=4,
                                      space="PSUM")
                        for (ps_, c0) in ((pr, 0), (pz, 200)):
                            nc.tensor.matmul(ps_[:], lhsT=msg_l,
                                             rhs=wx[s][:, c0:c0 + 200],
                                             start=True, stop=False)
                            nc.tensor.matmul(ps_[:], lhsT=ha,
                                             rhs=wh[s][0][:, c0:c0 + 200],
                                             start=False, stop=False)
                            nc.tensor.matmul(ps_[:], lhsT=hb,
                                             rhs=wh[s][1][:, c0:c0 + 200],
                                             start=False, stop=True)
                        nc.tensor.matmul(pnx[:], lhsT=msg_l,
                                         rhs=wx[s][:, 400:600],
                                         start=True, stop=True)
                        nc.tensor.matmul(pnh[:], lhsT=ha,
                                         rhs=wh[s][0][:, 400:600],
                                         start=True, stop=False)
                        nc.tensor.matmul(pnh[:], lhsT=hb,
                                         rhs=wh[s][1][:, 400:600],
                                         start=False, stop=True)

                        r_sb = p1.tile([P, 200], F32, tag="r", bufs=2)
                        nc.scalar.activation(r_sb[:], pr[:], AF.Sigmoid)
                        omz = p1.tile([P, 200], F32, tag="omz", bufs=2)
                        nc.scalar.activation(omz[:], pz[:], AF.Sigmoid,
                                             scale=-1.0)
                        rnh = p1.tile([P, 200], F32, tag="rnh", bufs=2)
                        nc.vector.tensor_tensor(out=rnh[:], in0=pnh[:],
                                                in1=r_sb[:], op=ALU.mult)
                        nsum = p1.tile([P, 200], F32, tag="nsum", bufs=2)
                        nc.vector.tensor_tensor(out=nsum[:], in0=pnx[:],
                                                in1=rnh[:], op=ALU.add)
                        n_sb = p1.tile([P, 200], F32, tag="nsb", bufs=2)
                        nc.scalar.activation(n_sb[:], nsum[:], AF.Tanh)
                        nmh = p1.tile([P, 200], F32, tag="nmh", bufs=2)
                        nc.vector.tensor_tensor(
                            out=nmh[:], in0=n_sb[:],
                            in1=g_sb[s][:, (htile0 + ch) * M:
                                        (htile0 + ch + 1) * M],
                            op=ALU.subtract)
                        nc.vector.tensor_tensor(
                            out=t_sb[:, ch * M:(ch + 1) * M], in0=nmh[:],
                            in1=omz[:], op=ALU.mult)
                        # prop
                        ppp = pp.tile([P, 200], F32, tag="gru", bufs=4,
                                      space="PSUM")
                        nc.tensor.matmul(ppp[:], lhsT=msg_l100,
                                         rhs=wprop[s][:, :],
                                         start=True, stop=True)
                        nc.scalar.activation(
                            t_sb[:, (ECH + ch) * M:(ECH + ch + 1) * M],
                            ppp[:], AF.Tanh)
                    # local T rows into inbox[0:2048]
                    nc.sync.dma_start(
                        inbox[s][0:2 * EC, :].rearrange("(k p) m -> p k m", p=P),
                        t_sb[:].rearrange("p (k m) -> p k m", m=M))

            # ================= phase 1.5: pack + A2A =================
            with tc.tile_pool(name="ph15", bufs=1) as p15:
                for s in "sg":
                    pk_idx = p15.tile([P, NP], I32, tag=f"pki_{s}")
                    nc.sync.dma_start(pk_idx[:], pack_idx_d[s][:, :])
                    pack_sb = p15.tile([P, NP * M], BF16, tag=f"pack_{s}")
                    for jj in range(NP):
                        nc.gpsimd.indirect_dma_start(
                            out=pack_sb[:, jj * M:(jj + 1) * M],
                            out_offset=None, in_=inbox[s][:, :],
                            in_offset=bass.IndirectOffsetOnAxis(
                                ap=pk_idx[:, jj:jj + 1], axis=0))
                    nc.sync.dma_start(
                        a2a_in[s][:, :].rearrange("(k p) m -> p k m", p=P),
                        pack_sb[:].rearrange("p (k m) -> p k m", m=M))
                    nc.gpsimd.collective_compute(
                        "AllToAll", ALU.bypass,
                        replica_groups=[list(range(C))],
                        ins=[a2a_in[s][:, :]],
                        outs=[inbox[s][2 * EC:IB, :]])

            # ================= phase 2: X gather + segsum + base =========
            with tc.tile_pool(name="ph2", bufs=1) as p2:
                for s in "sg":
                    xg_i = p2.tile([P, OT * Bx], I32, tag=f"xgi_{s}")
                    nc.sync.dma_start(xg_i[:], xg_idx_d[s][:, :])
                    sel_sb = p2.tile([P, OT * Bx * P], BF16, tag=f"sel_{s}")
                    nc.sync.dma_start(sel_sb[:], sel_d[s][:, :])
                    xg_sb = p2.tile([P, OT * Bx * M], BF16, tag=f"xg_{s}")
                    for jj in range(OT * Bx):
                        nc.gpsimd.indirect_dma_start(
                            out=xg_sb[:, jj * M:(jj + 1) * M],
                            out_offset=None, in_=inbox[s][:, :],
                            in_offset=bass.IndirectOffsetOnAxis(
                                ap=xg_i[:, jj:jj + 1], axis=0))
                    for o in range(OT):
                        dps = pp.tile([P, 200], F32, tag="gru", bufs=4,
                                      space="PSUM")
                        for b in range(Bx):
                            ob = o * Bx + b
                            nc.tensor.matmul(
                                dps[:], lhsT=sel_sb[:, ob * P:(ob + 1) * P],
                                rhs=xg_sb[:, ob * M:(ob + 1) * M],
                                start=(b == 0), stop=(b == Bx - 1))
                        nc.vector.tensor_tensor(
                            out=mem2[s][:, o * M:(o + 1) * M],
                            in0=g_sb[s][:, o * M:(o + 1) * M],
                            in1=dps[:], op=ALU.add)

            # ================= phase 3: score =================
            with tc.tile_pool(name="ph3", bufs=1) as p3:
                m2_fm = {}
                for s in "sg":
                    fa = p3.tile([100, S], F32, name=f"m2f_{s}a")
                    nb_ = 101 if s == "g" else 100
                    fb = p3.tile([nb_, S], F32, name=f"m2f_{s}b")
                    if s == "g":
                        nc.sync.dma_start(fb[100:101, :], ones_d[:, 0:S])
                    for j in range(OT):
                        for half, dstt in ((0, fa), (1, fb)):
                            pt = pp.tile([100, P], F32, tag="tr", bufs=2,
                                         space="PSUM")
                            nc.tensor.transpose(
                                out=pt[:],
                                in_=mem2[s][:, j * M + half * 100:
                                            j * M + half * 100 + 100],
                                identity=ident[:])
                            evac(dstt[0:100, j * P:(j + 1) * P], pt[:])
                    m2_fm[s] = (fa, fb)

                mm2_fm = (p3.tile([100, S], F32, name="mm2a"),
                          p3.tile([100, S], F32, name="mm2b"))
                rhs2 = [m2_fm["s"][0], m2_fm["s"][1], m2_fm["g"][0],
                        m2_fm["g"][1]]
                for mo in range(2):
                    for bch in range(S // 512):
                        bsl = slice(bch * 512, (bch + 1) * 512)
                        pm = pp.tile([100, 512], F32, tag="mm", bufs=2,
                                     space="PSUM")
                        for kc in range(4):
                            rt = rhs2[kc]
                            nr = rt.shape[0]
                            nc.tensor.matmul(
                                pm[:],
                                lhsT=wm[kc][0:nr, mo * 100:mo * 100 + 100],
                                rhs=rt[0:nr, bsl],
                                start=(kc == 0), stop=(kc == 3))
                        nc.scalar.activation(mm2_fm[mo][:, bsl], pm[:], AF.Tanh)

                # projections a = am@Ws, b = bm@Wg, c = cm@Wg  (form B)
                abc = []
                for blk, wt in ((0, ws), (1, wg), (2, wg)):
                    dst = p3.tile([P, ECH * 100], F32, name=f"abc{blk}")
                    for ch in range(ECH):
                        csl = slice(blk * EC + ch * P, blk * EC + (ch + 1) * P)
                        pm2 = pp.tile([P, 100], F32, tag="tr", bufs=2,
                                      space="PSUM")
                        nc.tensor.matmul(pm2[:], lhsT=mm2_fm[0][:, csl],
                                         rhs=wt[0][:, :], start=True,
                                         stop=False)
                        nc.tensor.matmul(pm2[:], lhsT=mm2_fm[1][:, csl],
                                         rhs=wt[1][:, :], start=False,
                                         stop=True)
                        evac(dst[:, ch * 100:(ch + 1) * 100], pm2[:])
                    abc.append(dst)

                res_sb = p3.tile([P, 2 * ECH], F32, name="res")
                for row, other in ((0, 1), (1, 2)):
                    for ch in range(ECH):
                        prod = p3.tile([P, 100], F32, tag="prod", bufs=2)
                        nc.vector.tensor_tensor(
                            out=prod[:],
                            in0=abc[0][:, ch * 100:(ch + 1) * 100],
                            in1=abc[other][:, ch * 100:(ch + 1) * 100],
                            op=ALU.mult)
                        red = p3.tile([P, 1], F32, tag="red", bufs=2)
                        nc.vector.tensor_reduce(red[:], prod[:],
                                                axis=mybir.AxisListType.X,
                                                op=ALU.add)
                        nc.scalar.activation(
                            res_sb[:, row * ECH + ch:row * ECH + ch + 1],
                            red[:], AF.Sigmoid)
                for row in range(2):
                    nc.sync.dma_start(
                        out_d[row, :].rearrange("(k p) -> p k", p=P),
                        res_sb[:, row * ECH:(row + 1) * ECH])

    nc.compile()
    return nc


_CACHED_NC = None


def kernel(**inputs):
    global _CACHED_NC, LAST_RESULT
    pp_ = _prep(inputs)

    mem_s = np.ascontiguousarray(np.asarray(inputs["memory_s"], dtype=np.float32))
    mem_g = np.ascontiguousarray(np.asarray(inputs["memory_g"], dtype=np.float32))
    ef_tab = np.ascontiguousarray(np.asarray(inputs["edge_feats"], dtype=np.float32))
    ones = np.ones((1, 4096), dtype=np.float32)

    import ml_dtypes
    in_maps = []
    for c in range(C):
        im = {
            "mem_s": mem_s, "mem_g": mem_g, "ef_tab": ef_tab,
            "slots_idx": pp_["slots_idx"][c],
            "ef_idx": pp_["ef_idx"][c],
            "Wmerge": pp_["Wmerge_d"], "Wmsg": pp_["Wmsg_d"],
            "Ws": pp_["Ws_d"], "Wg": pp_["Wg_d"], "ones": ones,
            "out": None,
        }
        del im["out"]
        for s in "sg":
            im[f"pack_idx_{s}"] = pp_["pack_idx"][s][c]
            im[f"xg_idx_{s}"] = pp_["xg_idx"][s][c]
            im[f"sel_{s}"] = pp_["sel"][s][c].astype(ml_dtypes.bfloat16)
            im[f"Wx_{s}"] = pp_[f"Wx_{s}_d"]
            im[f"Wh_{s}"] = pp_[f"Wh_{s}_d"]
            im[f"Wprop_{s}"] = pp_[f"Wprop_{s}_d"]
        in_maps.append(im)

    if _CACHED_NC is None:
        _CACHED_NC = _build()
    res = run_bass_kernel_spmd(_CACHED_NC, in_maps, core_ids=list(range(C)),
                               trace=TRACE)
    LAST_RESULT = res
    pos = np.concatenate([res.results[c]["out"][0] for c in range(C)])
    neg = np.concatenate([res.results[c]["out"][1] for c in range(C)])
    return (pos.astype(np.float32), neg.astype(np.float32))


# revision 4
# speedup vs baseline: 1.6749x; 1.6749x over previous
"""Trainium2 Bass kernel for nn_DGNN (TGN-style dual-memory GNN message passing).

Strategy (8 NeuronCores, SPMD):
  - Edges sharded across cores (1024/core). Per-core "slots" = [src; dst; neg]
    (3072 rows) are the only memory rows whose updated values the output needs.
  - Each core: indirect-gathers memory/edge-feature rows, computes merges,
    messages, GRU deltas and neighbor-prop vectors (the "T table") for its own
    edges; T rows needed by other cores are exchanged with one AllToAll per
    side (local rows bypass the collective entirely).
  - Scatter-add is a budgeted segment-sum: gather contribution rows from the
    inbox in slot order, multiply by host-built Sel blocks on the TensorEngine
    (PSUM accumulation), add the gathered base memory rows.
  - Scores: merge + bilinear projections + row-dot + sigmoid, all on-device.

Compute dtype is bf16 on the TensorEngine (f32 PSUM accumulation, f32 base
memory rows). Host side does index routing only (is_last flags, decay weights
folded into Sel, A2A packing plans); tensor math and memory-row traffic is
on-device.
"""
import sys

sys.path.insert(0, "/opt/trn_rl_repo")

import numpy as np

import concourse.bass as bass
import concourse.tile as tile
from concourse import bacc, mybir
from concourse.bass_utils import run_bass_kernel_spmd
from concourse.masks import make_identity

F32 = mybir.dt.float32
BF16 = mybir.dt.bfloat16
I32 = mybir.dt.int32
AF = mybir.ActivationFunctionType
ALU = mybir.AluOpType

N, M, DM, DE, E, K, NE = 200000, 200, 100, 172, 8192, 10, 400000
TAU = 2.0
C = 8
EC = E // C          # 1024 edges per core
S = 3 * EC           # 3072 slots per core
OT = S // 128        # 24 output slot tiles
ECH = EC // 128      # 8 edge chunks
P = 128
M2 = 2 * M           # combined (mem_s|mem_g) row width

TRACE = False
LAST_RESULT = None

# fixed (input-independent) exchange/budget sizes; host prep asserts they fit
NSP = 192            # A2A rows per (owner, receiver) pair, off-diagonal
NP = (C * NSP) // P  # pack gather column count (12)
IB = 2 * EC + C * NSP  # inbox rows: local T table + A2A output
# X-tile budget per output slot tile: hot region (own-endpoint slots, which
# receive the GRU deltas) gets 2 tiles, the rest 1.
BXT = {"s": [2] * 8 + [1] * 16, "g": [1] * 8 + [2] * 8 + [1] * 8}
XOFF = {s: np.concatenate([[0], np.cumsum(BXT[s])]).astype(int) for s in "sg"}
NX = {s: int(XOFF[s][-1]) for s in "sg"}  # 32 each


# --------------------------------------------------------------------------
# host-side preparation (index routing)
# --------------------------------------------------------------------------

def _is_last(nodes):
    order = np.arange(E, dtype=np.int64)
    last_occ = np.full(N, -1, dtype=np.int64)
    np.maximum.at(last_occ, nodes, order)
    return last_occ[nodes] == order


def _prep(inputs):
    src = np.asarray(inputs["source_nodes"]).astype(np.int64)
    dst = np.asarray(inputs["destination_nodes"]).astype(np.int64)
    neg = np.asarray(inputs["negative_nodes"]).astype(np.int64)
    eidx = np.asarray(inputs["edge_idxs"]).astype(np.int64)
    times = np.asarray(inputs["edge_times"]).astype(np.float64)
    nbrs = np.asarray(inputs["neighbors"]).astype(np.int64)
    lu = {
        "s": np.asarray(inputs["last_update_s"]).astype(np.float64),
        "g": np.asarray(inputs["last_update_g"]).astype(np.float64),
    }
    endp = {"s": src, "g": dst}
    out = {}

    slots = np.zeros((C, S), dtype=np.int64)
    for c in range(C):
        slots[c] = np.concatenate(
            [src[c * EC:(c + 1) * EC], dst[c * EC:(c + 1) * EC],
             neg[c * EC:(c + 1) * EC]])
    out["slots"] = slots
    out["slots_idx"] = np.transpose(
        slots.reshape(C, OT, P), (0, 2, 1)).astype(np.int32).copy()
    out["ef_idx"] = np.transpose(
        eidx.reshape(C, ECH, P), (0, 2, 1)).astype(np.int32).copy()

    flat_nodes = slots.reshape(-1)
    sort_perm = np.argsort(flat_nodes, kind="stable")
    sorted_nodes = flat_nodes[sort_perm]

    is_last = {"s": _is_last(src), "g": _is_last(dst)}

    sides = {}
    for side in ("s", "g"):
        nodes = endp[side]
        le = np.where(is_last[side])[0]
        nb = nbrs[nodes[le]]
        dt = times[le, None] - lu[side][nb]
        decay = np.exp(-np.clip(dt, 0.0, 50.0) / TAU)
        edge = np.concatenate([le, np.repeat(le, K)])
        kind = np.concatenate([np.zeros(len(le), np.int64),
                               np.ones(len(le) * K, np.int64)])
        tgt = np.concatenate([nodes[le], nb.reshape(-1)])
        w = np.concatenate([np.ones(len(le)), decay.reshape(-1)])

        lo = np.searchsorted(sorted_nodes, tgt, side="left")
        hi = np.searchsorted(sorted_nodes, tgt, side="right")
        cnt = hi - lo
        keep = cnt > 0
        lo, cnt = lo[keep], cnt[keep]
        edge, kind, w = edge[keep], kind[keep], w[keep]
        reps = np.repeat(np.arange(len(lo)), cnt)
        pos = (np.arange(len(reps)) -
               np.repeat(np.concatenate([[0], np.cumsum(cnt)[:-1]]), cnt))
        gslot = sort_perm[lo[reps] + pos]
        sides[side] = dict(edge=edge[reps], kind=kind[reps], w=w[reps],
                           core=gslot // S, lslot=gslot % S)

    # A2A pack lists (off-diagonal only)
    pair_rows = {}
    nsp_need = 0
    for side in ("s", "g"):
        sd = sides[side]
        owner = sd["edge"] // EC
        trow = sd["kind"] * EC + (sd["edge"] % EC)
        for o in range(C):
            for r in range(C):
                if o == r:
                    pair_rows[(side, o, r)] = np.zeros(0, dtype=np.int64)
                    continue
                m = (owner == o) & (sd["core"] == r)
                rows = np.unique(trow[m])
                pair_rows[(side, o, r)] = rows
                nsp_need = max(nsp_need, len(rows))
    assert nsp_need <= NSP, f"NSP {NSP} too small, need {nsp_need}"

    pack_idx = {}
    for side in ("s", "g"):
        pk = np.zeros((C, C * NSP), dtype=np.int32)
        for o in range(C):
            for r in range(C):
                rows = pair_rows[(side, o, r)]
                pk[o, r * NSP:r * NSP + len(rows)] = rows
        pack_idx[side] = np.transpose(
            pk.reshape(C, NP, P), (0, 2, 1)).copy()
    out["pack_idx"] = pack_idx

    xg_idx = {k: np.zeros((C, P, NX[k]), dtype=np.int32) for k in "sg"}
    sel = {k: np.zeros((C, P, NX[k] * P), dtype=np.float32) for k in "sg"}
    for side in ("s", "g"):
        sd = sides[side]
        owner = sd["edge"] // EC
        trow = sd["kind"] * EC + (sd["edge"] % EC)
        xoff = XOFF[side]
        for r in range(C):
            m = sd["core"] == r
            ow, tr = owner[m], trow[m]
            ls, ww = sd["lslot"][m], sd["w"][m]
            ib = np.zeros(len(ow), dtype=np.int64)
            for o in range(C):
                mo = ow == o
                if o == r:
                    ib[mo] = tr[mo]
                else:
                    rows = pair_rows[(side, o, r)]
                    ib[mo] = 2 * EC + o * NSP + np.searchsorted(rows, tr[mo])
            otile = ls // P
            order = np.argsort(otile, kind="stable")
            ib, ot_, tt, ww = ib[order], otile[order], (ls % P)[order], ww[order]
            for o in range(OT):
                mm = ot_ == o
                ibo, tto, wwo = ib[mm], tt[mm], ww[mm]
                assert len(ibo) <= BXT[side][o] * P, \
                    f"budget too small: side {side} tile {o}: {len(ibo)}"
                j = np.arange(len(ibo))
                b, i = j // P, j % P
                xg_idx[side][r, i, xoff[o] + b] = ibo
                np.add.at(sel[side][r], (i, (xoff[o] + b) * P + tto), wwo)
    out["xg_idx"] = xg_idx
    out["sel"] = sel

    W_merge = np.asarray(inputs["W_merge"], dtype=np.float32)
    b_merge = np.asarray(inputs["b_merge"], dtype=np.float32)
    W_msg = np.asarray(inputs["W_msg"], dtype=np.float32)
    b_msg = np.asarray(inputs["b_msg"], dtype=np.float32)
    out["Wmerge_d"] = np.concatenate([W_merge, b_merge[None, :]], axis=0)
    out["Wmsg_d"] = np.concatenate([W_msg, b_msg[None, :]], axis=0)
    for side in ("s", "g"):
        Wx = np.asarray(inputs[f"Wx_{side}"], dtype=np.float32)
        bg = np.asarray(inputs[f"bg_{side}"], dtype=np.float32)
        out[f"Wx_{side}_d"] = np.concatenate([Wx, bg[None, :]], axis=0)
        out[f"Wh_{side}_d"] = np.asarray(inputs[f"Wh_{side}"], dtype=np.float32)
        out[f"Wprop_{side}_d"] = np.asarray(inputs[f"W_prop_{side}"],
                                            dtype=np.float32)
    out["Ws_d"] = np.asarray(inputs["W_s"], dtype=np.float32)
    out["Wg_d"] = np.asarray(inputs["W_g"], dtype=np.float32)
    return out


# --------------------------------------------------------------------------
# device kernel
# --------------------------------------------------------------------------

def _build():
    nc = bacc.Bacc("TRN2", target_bir_lowering=False, debug=False,
                   num_devices=C)

    mem_d = nc.dram_tensor("mem_cat", [N, M2], F32, kind="ExternalInput")
    ef_d = nc.dram_tensor("ef_tab", [NE, DE], F32, kind="ExternalInput")
    slots_idx_d = nc.dram_tensor("slots_idx", [P, OT], I32, kind="ExternalInput")
    ef_idx_d = nc.dram_tensor("ef_idx", [P, ECH], I32, kind="ExternalInput")
    pack_idx_d = {s: nc.dram_tensor(f"pack_idx_{s}", [P, NP], I32,
                                    kind="ExternalInput") for s in "sg"}
    xg_idx_d = {s: nc.dram_tensor(f"xg_idx_{s}", [P, NX[s]], I32,
                                  kind="ExternalInput") for s in "sg"}
    sel_d = {s: nc.dram_tensor(f"sel_{s}", [P, NX[s] * P], BF16,
                               kind="ExternalInput") for s in "sg"}
    wm_d = nc.dram_tensor("Wmerge", [401, M], BF16, kind="ExternalInput")
    wmsg_d = nc.dram_tensor("Wmsg", [573, DM], BF16, kind="ExternalInput")
    wx_d = {s: nc.dram_tensor(f"Wx_{s}", [101, 600], BF16, kind="ExternalInput")
            for s in "sg"}
    wh_d = {s: nc.dram_tensor(f"Wh_{s}", [200, 600], BF16, kind="ExternalInput")
            for s in "sg"}
    wprop_d = {s: nc.dram_tensor(f"Wprop_{s}", [100, 200], BF16,
                                 kind="ExternalInput") for s in "sg"}
    ws_d = nc.dram_tensor("Ws", [200, 100], BF16, kind="ExternalInput")
    wg_d = nc.dram_tensor("Wg", [200, 100], BF16, kind="ExternalInput")
    ones_d = nc.dram_tensor("ones", [1, 4096], BF16, kind="ExternalInput")
    out_d = nc.dram_tensor("out", [2, EC], F32, kind="ExternalOutput")

    with tile.TileContext(nc) as tc:
        evac_ctr = [0]

        def evac(dst_ap, src_ap):
            # alternate PSUM->SBUF evacuation between DVE and ACT
            if evac_ctr[0] % 2 == 0:
                nc.vector.tensor_copy(dst_ap, src_ap)
            else:
                nc.scalar.copy(dst_ap, src_ap)
            evac_ctr[0] += 1

        with (
            tc.tile_pool(name="const", bufs=1) as cp,
            tc.tile_pool(name="psum", bufs=1, space="PSUM") as pp,
            tc.tile_pool(name="dram", bufs=1, space="DRAM") as dp,
            tc.tile_pool(name="glob", bufs=1) as gp,
        ):
            # ---------------- constants / weights ----------------
            ident = cp.tile([P, P], BF16)
            make_identity(nc, ident[:])
            ones = cp.tile([1, 4096], BF16)
            nc.sync.dma_start(ones[:1, :], ones_d[:, :])

            def load_rows(dram, r0, r1, cols, name):
                t = cp.tile([r1 - r0, cols], BF16, name=name)
                nc.sync.dma_start(t[:, :], dram[r0:r1, :])
                return t

            wm = [load_rows(wm_d, kc * 100, kc * 100 + 100 + (kc == 3), M,
                            f"wm{kc}") for kc in range(4)]
            wmsg = [load_rows(wmsg_d, kc * 100,
                              min(kc * 100 + 100, 572) + (kc == 5), DM,
                              f"wmsg{kc}") for kc in range(6)]
            wx = {s: load_rows(wx_d[s], 0, 101, 600, f"wx{s}") for s in "sg"}
            wh = {s: [load_rows(wh_d[s], kc * 100, kc * 100 + 100, 600,
                                f"wh{s}{kc}") for kc in range(2)] for s in "sg"}
            wprop = {s: load_rows(wprop_d[s], 0, 100, 200, f"wprop{s}")
                     for s in "sg"}
            ws = [load_rows(ws_d, kc * 100, kc * 100 + 100, 100, f"ws{kc}")
                  for kc in range(2)]
            wg = [load_rows(wg_d, kc * 100, kc * 100 + 100, 100, f"wg{kc}")
                  for kc in range(2)]

            slots_idx = cp.tile([P, OT], I32)
            nc.sync.dma_start(slots_idx[:], slots_idx_d[:, :])
            ef_idx = cp.tile([P, ECH], I32)
            nc.sync.dma_start(ef_idx[:], ef_idx_d[:, :])

            # ---------------- slot-row gathers (combined s|g table) -----
            g_cat = gp.tile([P, OT * M2], F32, name="g_cat")
            for j in range(OT):
                nc.gpsimd.indirect_dma_start(
                    out=g_cat[:, j * M2:(j + 1) * M2], out_offset=None,
                    in_=mem_d[:, :],
                    in_offset=bass.IndirectOffsetOnAxis(
                        ap=slots_idx[:, j:j + 1], axis=0))

            def gsl(s, j):
                # f32 memory rows of table s for slot tile j
                off = 0 if s == "s" else M
                return g_cat[:, j * M2 + off:j * M2 + off + M]

            inbox = {s: dp.tile([IB, M], BF16, name=f"inbox_{s}") for s in "sg"}
            a2a_in = {s: dp.tile([C * NSP, M], BF16, name=f"a2a_in_{s}")
                      for s in "sg"}
            mem2 = {s: gp.tile([P, OT * M], BF16, name=f"mem2_{s}") for s in "sg"}

            # ================= phase 1: per-edge compute =================
            with tc.tile_pool(name="ph1", bufs=1) as p1:
                # bf16 copy of src/dst slot rows (for transposes)
                g16 = p1.tile([P, 2 * ECH * M2], BF16)
                for j in range(2 * ECH):
                    evac(g16[:, j * M2:(j + 1) * M2],
                         g_cat[:, j * M2:(j + 1) * M2])

                ef_t = p1.tile([P, ECH * M], F32)
                nc.gpsimd.memset(ef_t[:], 0.0)
                for j in range(ECH):
                    nc.gpsimd.indirect_dma_start(
                        out=ef_t[:, j * M:j * M + DE], out_offset=None,
                        in_=ef_d[:, :],
                        in_offset=bass.IndirectOffsetOnAxis(
                            ap=ef_idx[:, j:j + 1], axis=0))
                ef16 = p1.tile([P, ECH * M], BF16)
                for j in range(ECH):
                    evac(ef16[:, j * M:(j + 1) * M], ef_t[:, j * M:(j + 1) * M])

                # FM transposes (bf16)
                g_fm = {}
                for s in "sg":
                    off = 0 if s == "s" else M
                    fa = p1.tile([100, 2 * EC], BF16, name=f"gfm_{s}a")
                    nb_ = 101 if s == "g" else 100
                    fb = p1.tile([nb_, 2 * EC], BF16, name=f"gfm_{s}b")
                    if s == "g":
                        nc.sync.dma_start(fb[100:101, :], ones_d[:, 0:2 * EC])
                    for j in range(2 * ECH):
                        for half, dstt in ((0, fa), (1, fb)):
                            pt = pp.tile([100, P], BF16, tag="tr", bufs=2,
                                         space="PSUM")
                            nc.tensor.transpose(
                                out=pt[:],
                                in_=g16[:, j * M2 + off + half * 100:
                                        j * M2 + off + half * 100 + 100],
                                identity=ident[:])
                            evac(dstt[0:100, j * P:(j + 1) * P], pt[:])
                    g_fm[s] = (fa, fb)

                ef_fm_a = p1.tile([100, EC], BF16)
                ef_fm_b = p1.tile([73, EC], BF16)
                nc.sync.dma_start(ef_fm_b[72:73, :], ones_d[:, 0:EC])
                for j in range(ECH):
                    pt = pp.tile([100, P], BF16, tag="tr", bufs=2, space="PSUM")
                    nc.tensor.transpose(out=pt[:],
                                        in_=ef16[:, j * M:j * M + 100],
                                        identity=ident[:])
                    evac(ef_fm_a[:, j * P:(j + 1) * P], pt[:])
                    pt2 = pp.tile([72, P], BF16, tag="tr", bufs=2, space="PSUM")
                    nc.tensor.transpose(out=pt2[:],
                                        in_=ef16[:, j * M + 100:j * M + DE],
                                        identity=ident[:])
                    evac(ef_fm_b[0:72, j * P:(j + 1) * P], pt2[:])

                # ---- merges (form A): mm_fm = tanh(Wm.T @ [g_s; g_g]) ----
                mm_fm = (p1.tile([100, 2 * EC], BF16, name="mmfa"),
                         p1.tile([100, 2 * EC], BF16, name="mmfb"))
                rhs_merge = [g_fm["s"][0], g_fm["s"][1], g_fm["g"][0],
                             g_fm["g"][1]]
                for mo in range(2):
                    for bch in range(2 * EC // 512):
                        bsl = slice(bch * 512, (bch + 1) * 512)
                        pm = pp.tile([100, 512], F32, tag="mm", bufs=2,
                                     space="PSUM")
                        for kc in range(4):
                            rt = rhs_merge[kc]
                            nr = rt.shape[0]
                            nc.tensor.matmul(
                                pm[:], lhsT=wm[kc][0:nr, mo * 100:mo * 100 + 100],
                                rhs=rt[0:nr, bsl],
                                start=(kc == 0), stop=(kc == 3))
                        nc.scalar.activation(mm_fm[mo][:, bsl], pm[:], AF.Tanh)

                # ---- messages (form A): relu(Wmsg.T @ [x_m; y_m; ef]) ----
                msg_fm = {}
                for s in "sg":
                    mf = p1.tile([101, EC], BF16, name=f"msg_{s}")
                    nc.sync.dma_start(mf[100:101, :], ones_d[:, 0:EC])
                    first, second = (0, EC) if s == "s" else (EC, 0)
                    for bch in range(EC // 512):
                        b0 = bch * 512
                        bsl = slice(b0, b0 + 512)
                        pm = pp.tile([100, 512], F32, tag="mm", bufs=2,
                                     space="PSUM")
                        rhss = [
                            mm_fm[0][:, first + b0:first + b0 + 512],
                            mm_fm[1][:, first + b0:first + b0 + 512],
                            mm_fm[0][:, second + b0:second + b0 + 512],
                            mm_fm[1][:, second + b0:second + b0 + 512],
                            ef_fm_a[:, bsl], ef_fm_b[:, bsl],
                        ]
                        for kc in range(6):
                            nr = rhss[kc].shape[0]
                            nc.tensor.matmul(pm[:], lhsT=wmsg[kc][0:nr, :],
                                             rhs=rhss[kc],
                                             start=(kc == 0), stop=(kc == 5))
                        nc.scalar.activation(mf[0:100, bsl], pm[:], AF.Relu)
                    msg_fm[s] = mf

                # ---- GRU + prop -> T table (bf16), DMA to inbox ----
                for s in "sg":
                    t_sb = p1.tile([P, 2 * ECH * M], BF16, name=f"t_{s}")
                    hcol = 0 if s == "s" else EC
                    htile0 = 0 if s == "s" else ECH
                    fa, fb = g_fm[s]
                    for ch in range(ECH):
                        mslc = slice(ch * P, (ch + 1) * P)
                        hslc = slice(hcol + ch * P, hcol + (ch + 1) * P)
                        msg_l = msg_fm[s][0:101, mslc]
                        msg_l100 = msg_fm[s][0:100, mslc]
                        ha = fa[0:100, hslc]
                        hb = fb[0:100, hslc]
                        prz = pp.tile([P, 400], F32, tag="gru", bufs=4,
                                      space="PSUM")
                        pnx = pp.tile([P, 200], F32, tag="gru", bufs=4,
                                      space="PSUM")
                        pnh = pp.tile([P, 200], F32, tag="gru", bufs=4,
                                      space="PSUM")
                        nc.tensor.matmul(prz[:], lhsT=msg_l,
                                         rhs=wx[s][:, 0:400],
                                         start=True, stop=False)
                        nc.tensor.matmul(prz[:], lhsT=ha,
                                         rhs=wh[s][0][:, 0:400],
                                         start=False, stop=False)
                        nc.tensor.matmul(prz[:], lhsT=hb,
                                         rhs=wh[s][1][:, 0:400],
                                         start=False, stop=True)
                        nc.tensor.matmul(pnx[:], lhsT=msg_l,
                                         rhs=wx[s][:, 400:600],
                                         start=True, stop=True)
                        nc.tensor.matmul(pnh[:], lhsT=ha,
                                         rhs=wh[s][0][:, 400:600],
                                         start=True, stop=False)
                        nc.tensor.matmul(pnh[:], lhsT=hb,
                                         rhs=wh[s][1][:, 400:600],
                                         start=False, stop=True)

                        r_sb = p1.tile([P, 200], F32, tag="r", bufs=2)
                        nc.scalar.activation(r_sb[:], prz[:, 0:200], AF.Sigmoid)
                        omz = p1.tile([P, 200], F32, tag="omz", bufs=2)
                        nc.scalar.activation(omz[:], prz[:, 200:400],
                                             AF.Sigmoid, scale=-1.0)
                        rnh = p1.tile([P, 200], F32, tag="rnh", bufs=2)
                        nc.vector.tensor_tensor(out=rnh[:], in0=pnh[:],
                                                in1=r_sb[:], op=ALU.mult)
                        nsum = p1.tile([P, 200], F32, tag="nsum", bufs=2)
                        nc.vector.tensor_tensor(out=nsum[:], in0=pnx[:],
                                                in1=rnh[:], op=ALU.add)
                        n_sb = p1.tile([P, 200], F32, tag="nsb", bufs=2)
                        nc.scalar.activation(n_sb[:], nsum[:], AF.Tanh)
                        nmh = p1.tile([P, 200], F32, tag="nmh", bufs=2)
                        nc.vector.tensor_tensor(
                            out=nmh[:], in0=n_sb[:],
                            in1=gsl(s, htile0 + ch),
                            op=ALU.subtract)
                        nc.vector.tensor_tensor(
                            out=t_sb[:, ch * M:(ch + 1) * M], in0=nmh[:],
                            in1=omz[:], op=ALU.mult)
                        # prop
                        ppp = pp.tile([P, 200], F32, tag="gru", bufs=4,
                                      space="PSUM")
                        nc.tensor.matmul(ppp[:], lhsT=msg_l100,
                                         rhs=wprop[s][:, :],
                                         start=True, stop=True)
                        nc.scalar.activation(
                            t_sb[:, (ECH + ch) * M:(ECH + ch + 1) * M],
                            ppp[:], AF.Tanh)
                    nc.sync.dma_start(
                        inbox[s][0:2 * EC, :].rearrange("(k p) m -> p k m", p=P),
                        t_sb[:].rearrange("p (k m) -> p k m", m=M))

            # ================= phase 1.5: pack + A2A =================
            with tc.tile_pool(name="ph15", bufs=1) as p15:
                for s in "sg":
                    pk_idx = p15.tile([P, NP], I32, tag=f"pki_{s}")
                    nc.sync.dma_start(pk_idx[:], pack_idx_d[s][:, :])
                    pack_sb = p15.tile([P, NP * M], BF16, tag=f"pack_{s}")
                    for jj in range(NP):
                        nc.gpsimd.indirect_dma_start(
                            out=pack_sb[:, jj * M:(jj + 1) * M],
                            out_offset=None, in_=inbox[s][:, :],
                            in_offset=bass.IndirectOffsetOnAxis(
                                ap=pk_idx[:, jj:jj + 1], axis=0))
                    nc.sync.dma_start(
                        a2a_in[s][:, :].rearrange("(k p) m -> p k m", p=P),
                        pack_sb[:].rearrange("p (k m) -> p k m", m=M))
                    nc.gpsimd.collective_compute(
                        "AllToAll", ALU.bypass,
                        replica_groups=[list(range(C))],
                        ins=[a2a_in[s][:, :]],
                        outs=[inbox[s][2 * EC:IB, :]])

            # ================= phase 2: X gather + segsum + base =========
            with tc.tile_pool(name="ph2", bufs=1) as p2:
                for s in "sg":
                    xg_i = p2.tile([P, NX[s]], I32, tag=f"xgi_{s}")
                    nc.sync.dma_start(xg_i[:], xg_idx_d[s][:, :])
                    sel_sb = p2.tile([P, NX[s] * P], BF16, tag=f"sel_{s}")
                    nc.sync.dma_start(sel_sb[:], sel_d[s][:, :])
                    xg_sb = p2.tile([P, NX[s] * M], BF16, tag=f"xg_{s}")
                    for jj in range(NX[s]):
                        nc.gpsimd.indirect_dma_start(
                            out=xg_sb[:, jj * M:(jj + 1) * M],
                            out_offset=None, in_=inbox[s][:, :],
                            in_offset=bass.IndirectOffsetOnAxis(
                                ap=xg_i[:, jj:jj + 1], axis=0))
                    xoff = XOFF[s]
                    for o in range(OT):
                        nb_ = BXT[s][o]
                        dps = pp.tile([P, 200], F32, tag="gru", bufs=4,
                                      space="PSUM")
                        for b in range(nb_):
                            ob = int(xoff[o]) + b
                            nc.tensor.matmul(
                                dps[:], lhsT=sel_sb[:, ob * P:(ob + 1) * P],
                                rhs=xg_sb[:, ob * M:(ob + 1) * M],
                                start=(b == 0), stop=(b == nb_ - 1))
                        nc.vector.tensor_tensor(
                            out=mem2[s][:, o * M:(o + 1) * M],
                            in0=gsl(s, o),
                            in1=dps[:], op=ALU.add)

            # ================= phase 3: score =================
            with tc.tile_pool(name="ph3", bufs=1) as p3:
                m2_fm = {}
                for s in "sg":
                    fa = p3.tile([100, S], BF16, name=f"m2f_{s}a")
                    nb_ = 101 if s == "g" else 100
                    fb = p3.tile([nb_, S], BF16, name=f"m2f_{s}b")
                    if s == "g":
                        nc.sync.dma_start(fb[100:101, :], ones_d[:, 0:S])
                    for j in range(OT):
                        for half, dstt in ((0, fa), (1, fb)):
                            pt = pp.tile([100, P], BF16, tag="tr", bufs=2,
                                         space="PSUM")
                            nc.tensor.transpose(
                                out=pt[:],
                                in_=mem2[s][:, j * M + half * 100:
                                            j * M + half * 100 + 100],
                                identity=ident[:])
                            evac(dstt[0:100, j * P:(j + 1) * P], pt[:])
                    m2_fm[s] = (fa, fb)

                mm2_fm = (p3.tile([100, S], BF16, name="mm2a"),
                          p3.tile([100, S], BF16, name="mm2b"))
                rhs2 = [m2_fm["s"][0], m2_fm["s"][1], m2_fm["g"][0],
                        m2_fm["g"][1]]
                for mo in range(2):
                    for bch in range(S // 512):
                        bsl = slice(bch * 512, (bch + 1) * 512)
                        pm = pp.tile([100, 512], F32, tag="mm", bufs=2,
                                     space="PSUM")
                        for kc in range(4):
                            rt = rhs2[kc]
                            nr = rt.shape[0]
                            nc.tensor.matmul(
                                pm[:],
                                lhsT=wm[kc][0:nr, mo * 100:mo * 100 + 100],
                                rhs=rt[0:nr, bsl],
                                start=(kc == 0), stop=(kc == 3))
                        nc.scalar.activation(mm2_fm[mo][:, bsl], pm[:], AF.Tanh)

                abc = []
                for blk, wt in ((0, ws), (1, wg), (2, wg)):
                    dst = p3.tile([P, ECH * 100], F32, name=f"abc{blk}")
                    for ch in range(ECH):
                        csl = slice(blk * EC + ch * P, blk * EC + (ch + 1) * P)
                        pm2 = pp.tile([P, 100], F32, tag="tr", bufs=2,
                                      space="PSUM")
                        nc.tensor.matmul(pm2[:], lhsT=mm2_fm[0][:, csl],
                                         rhs=wt[0][:, :], start=True,
                                         stop=False)
                        nc.tensor.matmul(pm2[:], lhsT=mm2_fm[1][:, csl],
                                         rhs=wt[1][:, :], start=False,
                                         stop=True)
                        evac(dst[:, ch * 100:(ch + 1) * 100], pm2[:])
                    abc.append(dst)

                res_sb = p3.tile([P, 2 * ECH], F32, name="res")
                for row, other in ((0, 1), (1, 2)):
                    for ch in range(ECH):
                        prod = p3.tile([P, 100], F32, tag="prod", bufs=2)
                        nc.vector.tensor_tensor(
                            out=prod[:],
                            in0=abc[0][:, ch * 100:(ch + 1) * 100],
                            in1=abc[other][:, ch * 100:(ch + 1) * 100],
                            op=ALU.mult)
                        red = p3.tile([P, 1], F32, tag="red", bufs=2)
                        nc.vector.tensor_reduce(red[:], prod[:],
                                                axis=mybir.AxisListType.X,
                                                op=ALU.add)
                        nc.scalar.activation(
                            res_sb[:, row * ECH + ch:row * ECH + ch + 1],
                            red[:], AF.Sigmoid)
                for row in range(2):
                    nc.sync.dma_start(
                        out_d[row, :].rearrange("(k p) -> p k", p=P),
                        res_sb[:, row * ECH:(row + 1) * ECH])

    nc.compile()
    return nc


_CACHED_NC = None


def kernel(**inputs):
    global _CACHED_NC, LAST_RESULT
    import ml_dtypes
    pp_ = _prep(inputs)

    mem_cat = np.concatenate(
        [np.asarray(inputs["memory_s"], dtype=np.float32),
         np.asarray(inputs["memory_g"], dtype=np.float32)], axis=1)
    mem_cat = np.ascontiguousarray(mem_cat)
    ef_tab = np.ascontiguousarray(np.asarray(inputs["edge_feats"],
                                             dtype=np.float32))
    ones = np.ones((1, 4096), dtype=ml_dtypes.bfloat16)

    def b16(x):
        return np.ascontiguousarray(x.astype(ml_dtypes.bfloat16))

    in_maps = []
    for c in range(C):
        im = {
            "mem_cat": mem_cat, "ef_tab": ef_tab,
            "slots_idx": pp_["slots_idx"][c],
            "ef_idx": pp_["ef_idx"][c],
            "Wmerge": b16(pp_["Wmerge_d"]), "Wmsg": b16(pp_["Wmsg_d"]),
            "Ws": b16(pp_["Ws_d"]), "Wg": b16(pp_["Wg_d"]), "ones": ones,
        }
        for s in "sg":
            im[f"pack_idx_{s}"] = pp_["pack_idx"][s][c]
            im[f"xg_idx_{s}"] = pp_["xg_idx"][s][c]
            im[f"sel_{s}"] = b16(pp_["sel"][s][c])
            im[f"Wx_{s}"] = b16(pp_[f"Wx_{s}_d"])
            im[f"Wh_{s}"] = b16(pp_[f"Wh_{s}_d"])
            im[f"Wprop_{s}"] = b16(pp_[f"Wprop_{s}_d"])
        in_maps.append(im)

    if _CACHED_NC is None:
        _CACHED_NC = _build()
    res = run_bass_kernel_spmd(_CACHED_NC, in_maps, core_ids=list(range(C)),
                               trace=TRACE)
    LAST_RESULT = res
    pos = np.concatenate([res.results[c]["out"][0] for c in range(C)])
    neg = np.concatenate([res.results[c]["out"][1] for c in range(C)])
    return (pos.astype(np.float32), neg.astype(np.float32))
